# revision 22
# baseline (speedup 1.0000x reference)
"""Trainium2 Bass kernel for nn_LiquidS4Layer (S4 DPLR forward).

y = causal_conv(u, K) + D*u, with K the length-L SSM kernel computed from
small DPLR params (Lambda, P, B, C, step).

The tiny parameter pipeline (N=64 modes -> K and the chunk-recurrence
matrices, O(N^2 L) flops) runs on host in fp64 numpy; the memory-bound
convolution over u (BH*L = 16M elements) runs on the NeuronCores.

Device algorithm, per core over 512 of the 4096 batch rows, chunk Q=128:
  near field   y[i] += T0^T u[i]          (intra-chunk causal Toeplitz + D)
  direct       y[2k+1] += G0^T u[2k]      (adjacent-chunk Toeplitz block)
  far field    y[2k] += Wo^T h_k ; y[2k+1] += W1^T h_k
  recurrence   h_{k+1} = Phi2^T h_k + Et^T u[2k] + Mn^T u[2k+1]
with h the 2N=128-dim real-embedded SSM state per row.  All matmuls are
bf16 with fp32 PSUM accumulation, 512-wide moving operands (the PSUM
bank limit), stationary weights [128,128].

Engine layout (per trace analysis):
 - tensor: one uninterrupted matmul stream (8 warm-up mms on a memset
   tile sized to end when the first u piece lands, then 8 mms/pair);
   any idle gap >3.4us re-throttles the PE clock (HAM K=4/8), so the
   schedule keeps the PE back-to-back.
 - scalar: ONLY the chain-critical h copies (PSUM->SBUF bf16) + one
   early dummy copy to pre-load the lazy ACT table + the big weight-DMA
   config.  Keeping u-piece configs off scalar removes the 2.7us chain
   stall the previous layout had.
 - vector: all y casts (2x [128,512] per pair) + the warm-up memset.
 - sync (HWDGE): all 9 u-piece configs (it is otherwise idle).
 - gpsimd (SWDGE): small weight piece + y DMAs (2 pairs fused per DMA,
   4KB DRAM lines, halving output descriptors).
The state chain advances one pair ahead of its far-field use, so each
chain step (Phi mm -> scalar copy) has a full pair (~1.9us) of PE work
to hide under.

Sharding: u/y row-sharded over 8 cores (batch*channel parallel); the
small weight pack (9 x [128,128] bf16) is replicated; no collectives.
"""
import os
import numpy as np
import ml_dtypes
from contextlib import ExitStack

import concourse.bass as bass
import concourse.tile as tile
from concourse import mybir
from concourse.bass_utils import run_bass_kernel_spmd

F32 = mybir.dt.float32
BF16 = mybir.dt.bfloat16
NPBF16 = ml_dtypes.bfloat16

NCORES = 8
BH, L = 4096, 4096
BC = BH // NCORES       # 512 rows per core
N = 64                  # SSM state size
Q = 128                 # chunk length
NCH = L // Q            # 32 chunks
NPAIR = NCH // 2        # 16 chunk pairs

LAST_EXEC_NS = None
LAST_RESULTS = None


# --------------------------------------------------------------------------
# Host parameter pipeline (fp64): DPLR params -> K -> device weight pack
# --------------------------------------------------------------------------
def _host_weights(Lambda_re, Lambda_im, P_re, P_im, B_re, B_im, C_ri, D,
                  log_step):
    Lam = (np.asarray(Lambda_re, np.float64)
           + 1j * np.asarray(Lambda_im, np.float64)).reshape(N)
    P = (np.asarray(P_re, np.float64)
         + 1j * np.asarray(P_im, np.float64)).reshape(N)
    B = (np.asarray(B_re, np.float64)
         + 1j * np.asarray(B_im, np.float64)).reshape(N)
    C_ri = np.asarray(C_ri, np.float64).reshape(N, 2)
    C = C_ri[:, 0] + 1j * C_ri[:, 1]
    step = float(np.exp(np.asarray(log_step, np.float64).reshape(())))
    Dv = float(np.asarray(D, np.float64).reshape(()))

    # K via the reference's generating-function path (roots of unity + ifft)
    l = np.arange(L)
    Om = np.exp((-2j * np.pi) * (l / L))
    a0, a1 = np.conj(C), np.conj(P)
    b0, b1 = B, P
    g = (2.0 / step) * ((1.0 - Om) / (1.0 + Om))
    cc = 2.0 / (1.0 + Om)

    def cauchy(v):
        return (v[None, :] / (g[:, None] - Lam[None, :])).sum(-1)

    k00 = cauchy(a0 * b0)
    k01 = cauchy(a0 * b1)
    k10 = cauchy(a1 * b0)
    k11 = cauchy(a1 * b1)
    at_roots = cc * (k00 - k01 * (1.0 / (1.0 + k11)) * k10)
    K = np.fft.ifft(at_roots, L).real  # (L,) aliased causal kernel

    # State space: A = diag(Lam) - P P^H, bilinear discretization, and the
    # alias-corrected input vector Bp so that K[l] = Re(Ct @ Abar^l @ Bp).
    A = np.diag(Lam) - np.outer(P, np.conj(P))
    I = np.eye(N)
    inv = np.linalg.inv(I - (step / 2.0) * A)
    Abar = inv @ (I + (step / 2.0) * A)
    Bbar = inv @ (step * B)
    AL = np.linalg.matrix_power(Abar, L)
    Bp = np.linalg.solve(I - AL, Bbar)
    Ct = np.conj(C)

    # complex [hr; hi] block embedding
    def embed_mat(M):
        return np.block([[M.real, -M.imag], [M.imag, M.real]])

    def embed_vec(x):
        return np.concatenate([x.real, x.imag])

    # Wout[s, t]: y_t = Re(Ct A^t h);  W1 continues t in [128, 256)
    Wout = np.zeros((2 * N, Q))
    W1 = np.zeros((2 * N, Q))
    gt = Ct.copy()
    for t in range(Q):
        Wout[:N, t] = gt.real
        Wout[N:, t] = -gt.imag
        gt = gt @ Abar
    for t in range(Q):
        W1[:N, t] = gt.real
        W1[N:, t] = -gt.imag
        gt = gt @ Abar

    # Min[s', q] = embed(A^{128-q} Bp);  E[s', q] = embed(A^{256-q} Bp)
    cols = [None] * 257  # cols[e] = A^e Bp
    v = Abar @ Bp
    for e in range(1, 257):
        cols[e] = v
        v = Abar @ v
    Min_r = np.zeros((2 * N, Q))
    E_r = np.zeros((2 * N, Q))
    for q in range(Q):
        Min_r[:, q] = embed_vec(cols[128 - q])
        E_r[:, q] = embed_vec(cols[256 - q])

    A128 = np.linalg.matrix_power(Abar, 128)
    A256 = A128 @ A128
    Phi2 = embed_mat(A256)  # A^256
    # shifted projection maps for the chain-free prologue states h_2, h_3
    P2E, P2M = Phi2 @ E_r, Phi2 @ Min_r
    P4 = embed_mat(A256 @ A256)
    P4E, P4M = P4 @ E_r, P4 @ Min_r

    # Toeplitz slabs from K (lhsT layout [q, t])
    idx_t = np.arange(Q)[None, :]
    idx_q = np.arange(Q)[:, None]
    lag = idx_t - idx_q
    T0 = np.where(lag >= 0, K[np.clip(lag, 0, L - 1)], 0.0)
    T0 = T0 + Dv * np.eye(Q)
    G0 = K[128 + lag]

    # pack, lhsT convention (partition dim = contraction dim)
    pack = np.concatenate(
        [T0, G0, Wout, W1, Min_r.T, E_r.T, Phi2.T, P2E.T, P2M.T,
         P4E.T, P4M.T],
        axis=1)  # [128, 11*128]
    return np.ascontiguousarray(pack).astype(NPBF16)


# --------------------------------------------------------------------------
# Device program
# --------------------------------------------------------------------------
def build_program():
    nc = bass.Bass()
    dp = nc.declare_dram_parameter
    uT_d = dp("uT", [128, NCH * 512], BF16, isOutput=False)
    w_d = dp("W", [128, 11 * 128], BF16, isOutput=False)
    y_d = dp("y", [8 * 128, 2048], BF16, isOutput=True)
    with TileKernel(nc) as tk:
        tk.build(uT_d, w_d, y_d)
    _split_multi_waits(nc)
    return nc


def _split_multi_waits(nc):
    """This toolchain's walrus encodes at most one sync wait per (non-Drain)
    instruction.  Tile can emit several; hoist the extras onto standalone
    EventSemaphore wait instructions inserted just before, on the same
    engine (engines execute their stream in order, so this is equivalent)."""
    ctr = 0
    for f in nc.m.functions:
        for blk in f.blocks:
            out = []
            changed = False
            for inst in blk.instructions:
                si = inst.sync_info
                if si is None:
                    out.append(inst)
                    continue
                waits = list(si.on_wait)
                if len(waits) > 1:
                    # pick a non-DMA sem for the no-op update (the sim
                    # forbids foreign updates of in-flight DMA sems)
                    cands = [u for u in si.on_update] + [
                        w for w in waits if "DMA" not in w.ant_name]
                    for w in waits[:-1]:
                        ev = mybir.InstEventSemaphore(
                            name=f"I-wsplit-{ctr}", ins=[], outs=[])
                        ctr += 1
                        ev.engine = inst.engine
                        # zero-increment update: the sim requires >=1 update
                        # per instruction; +0 changes no semaphore value.
                        c = cands[0] if cands else w
                        up = mybir.SyncUpdate(
                            sync_type="semaphore", id=c.id, ant_name=c.ant_name,
                            update_mode="sem-add-imm", update_value=0,
                            update_reg=None)
                        ev.sync_info = mybir.SyncInfo(on_wait=[w], on_update=[up])
                        out.append(ev)
                    inst.sync_info = mybir.SyncInfo(
                        on_wait=[waits[-1]], on_update=list(si.on_update))
                    changed = True
                out.append(inst)
            if changed:
                blk.instructions = out
    return nc


class TileKernel:
    def __init__(self, nc):
        self.nc = nc
        self.ctx = ExitStack()
        self.tc = tile.TileContext(nc)

    def __enter__(self):
        self.ctx.__enter__()
        self.tc.__enter__()
        return self

    def __exit__(self, *a):
        self.ctx.__exit__(*a)   # release pools before the scheduler runs
        return self.tc.__exit__(*a)

    def pool(self, name, bufs=1, space="SBUF"):
        return self.ctx.enter_context(
            self.tc.tile_pool(name=name, bufs=bufs, space=space))

    def build(self, uT_d, w_d, y_d):
        nc = self.nc
        mm = nc.tensor.matmul
        v = nc.vector
        s = nc.scalar

        wp = self.pool("w", 1)
        up = self.pool("u", 1)
        hp = self.pool("h", 3)
        yp = self.pool("yt", 3)
        # 6 py banks (3 pairs in flight, so a pair's near-field mms never
        # wait on casts two pairs back) + 2 ph banks = all 8 PSUM banks;
        # the warm-up tile borrows from the py pool before pair 2 needs it.
        pyp = self.pool("py", 6, "PSUM")
        php = self.pool("ph", 2, "PSUM")

        Wt = wp.tile([128, 11 * 128], BF16, tag="Wt", name="Wt")
        # The whole weight pack as ONE piece on the scalar HWDGE ACT ring:
        # it transfers mostly alone (the u flood on the SP ring starts
        # ~1us later), landing ~10.4us — right when u piece 0 does — and
        # keeps the SP ring pure-u so every u piece lands ~0.8us earlier.
        # (Split variants lose: a second W piece round-robins against the
        # u flood at the SDMA engines and arrives microseconds late.)
        nc.scalar.dma_start(out=Wt[:], in_=w_d[:])
        (T0, G0, Wo, W1, Mn, Et, Ph, P2E, P2M, P4E, P4M) = (
            Wt[:, m * 128:(m + 1) * 128] for m in range(11))

        uT = up.tile([128, NCH, 4, 128], BF16, tag="uT", name="uT")
        # all u-piece configs on the otherwise-idle sync HWDGE, in
        # consumption order: four small lead pieces (2 chunks each) so the
        # prologue's chunks trickle in early, then 4-chunk pieces (4KB
        # DRAM lines halve the descriptor count).
        pieces = ([(2 * m, 2) for m in range(4)]
                  + [(8 + 4 * m, 4) for m in range(6)])
        for c0, nchunks in pieces:
            nc.sync.dma_start(out=uT[:, c0:c0 + nchunks, :, :],
                              in_=uT_d[:, c0 * 512:(c0 + nchunks) * 512])

        def uch(i):
            return uT[:, i, :, :]

        # warm-up scratch + ACT-table preload scratch.  The memset goes on
        # gpsimd — the earliest-booting engine — so the PE warm-up stream
        # starts as soon as possible: the HAM un-throttle fires ~3.4us
        # after sustained PE activity begins, so every ns earlier here is
        # one less ns of real work running at the cold 1.2GHz clock.
        wsrc = wp.tile([128, 256], BF16, tag="wsrc", name="wsrc")
        nc.gpsimd.memset(wsrc[:], 0.0)
        scr = wp.tile([128, 2], BF16, tag="scr", name="scr")
        s.copy(scr[:], wsrc[:, 0:2])     # lazy ACT table load, off the chain

        # PE warm-up on the memset tile: no DMA dependence, sized to bridge
        # until the first u piece lands, keeping the HAM activity window fed
        # from the earliest possible instruction.
        warm = pyp.tile([128, 256], F32, tag="py", name="warm")
        for _ in range(11):
            mm(warm[:], wsrc[:, 0:128], wsrc[:], start=True, stop=True)

        # ---- y output: 2 pairs fused per [128,2048] tile / DMA ----------
        # y_d row jj*128+t, col c4*512 + j*128 + b'  (c4 = chunk in quad)
        yt_cur = [None]

        def cast_pair(py_a, py_b, k, split_last=False):
            jj, half = divmod(k, 2)
            if half == 0:
                yt_cur[0] = yp.tile([128, 2048], BF16, tag="yt", name="yt")
            yt = yt_cur[0]
            o = half * 1024
            if split_last:
                # tail: halve the final cast latency across both copy
                # engines (the scalar chain is done by now) and issue the
                # halves on the two low-latency HWDGE rings so transfer
                # overlaps cast.
                v.tensor_copy(yt[:, o:o + 512], py_a[:])
                s.copy(yt[:, o + 512:o + 1024], py_b[:])
                nc.sync.dma_start(
                    out=y_d[jj * 128:(jj + 1) * 128, o:o + 512],
                    in_=yt[:, o:o + 512])
                nc.scalar.dma_start(
                    out=y_d[jj * 128:(jj + 1) * 128, o + 512:o + 1024],
                    in_=yt[:, o + 512:o + 1024])
            else:
                v.tensor_copy(yt[:, o:o + 512], py_a[:])
                v.tensor_copy(yt[:, o + 512:o + 1024], py_b[:])
                if half == 1:
                    nc.gpsimd.dma_start(
                        out=y_d[jj * 128:(jj + 1) * 128, :], in_=yt[:])

        # ---- software pipeline over 16 chunk pairs -------------------
        # pair 0 near fields (h_0 = 0, so no far field)
        py_a = pyp.tile([128, 512], F32, tag="py", name="py_a")
        py_b = pyp.tile([128, 512], F32, tag="py", name="py_b")
        mm(py_a[:], T0, uch(0), start=True, stop=True)
        mm(py_b[:], T0, uch(1), start=True, stop=False)
        mm(py_b[:], G0, uch(0), start=False, stop=True)
        cast_pair(py_a, py_b, 0)

        def state(terms, chain=None):
            # h = sum_m w_m^T u_cm (+ Phi^T h_prev first); bf16 copy on
            # scalar (the only thing scalar does, so the chain never queues)
            ph = php.tile([128, 512], F32, tag="ph", name="ph")[:]
            nterms = len(terms) + (1 if chain is not None else 0)
            i = 0
            if chain is not None:
                mm(ph, Ph, chain[:], start=True, stop=(nterms == 1))
                i = 1
            for m, (w, c) in enumerate(terms):
                mm(ph, w, uch(c), start=(i + m == 0),
                   stop=(i + m == nterms - 1))
            h = hp.tile([128, 512], BF16, tag="h", name="h")
            s.copy(h[:], ph)
            return h

        # chain-free prologue states
        h_cur = state([(Et, 0), (Mn, 1)])                    # h_1
        # pair 1 near fields
        py_a2 = pyp.tile([128, 512], F32, tag="py", name="py_a")
        py_b2 = pyp.tile([128, 512], F32, tag="py", name="py_b")
        mm(py_a2[:], T0, uch(2), start=True, stop=False)
        mm(py_b2[:], T0, uch(3), start=True, stop=False)
        mm(py_b2[:], G0, uch(2), start=False, stop=True)
        h_nxt = state([(P2E, 0), (P2M, 1), (Et, 2), (Mn, 3)])  # h_2
        py_a, py_b = py_a2, py_b2

        for k in range(1, NPAIR):
            last = (k == NPAIR - 1)
            # far fields of pair k FIRST (h_cur has long been ready) — the
            # PE queue is in-order, so putting the chain-dependent Phi mm
            # ahead of them head-of-line blocks the whole pipe whenever the
            # h copy is the least bit late (it is, during pipeline fill).
            mm(py_a[:], Wo, h_cur[:], start=False, stop=True)
            mm(py_b[:], W1, h_cur[:], start=False, stop=True)
            # advance the chain one pair ahead of use: h_{k+2}.
            # h_3 is built chain-free from shifted projection maps so the
            # serial chain only starts at h_4, by when the pipeline is deep
            # enough to hide each (Phi mm -> scalar copy) step.
            if k == 1:
                h_new = state([(P4E, 0), (P4M, 1), (P2E, 2), (P2M, 3),
                               (Et, 4), (Mn, 5)])
            elif k <= NPAIR - 3:
                h_new = state([(Et, 2 * k + 2), (Mn, 2 * k + 3)],
                              chain=h_nxt)
            # last TWO pairs take the split cast/DMA path: both copy
            # engines + both HWDGE rings, so the tail after the final
            # matmul is two short parallel pipes instead of one long one.
            cast_pair(py_a, py_b, k, split_last=(k >= NPAIR - 2))
            if not last:
                # near fields of pair k+1
                py_a = pyp.tile([128, 512], F32, tag="py", name="py_a")
                py_b = pyp.tile([128, 512], F32, tag="py", name="py_b")
                mm(py_a[:], T0, uch(2 * k + 2), start=True, stop=False)
                mm(py_b[:], T0, uch(2 * k + 3), start=True, stop=False)
                mm(py_b[:], G0, uch(2 * k + 2), start=False, stop=True)
                h_cur = h_nxt
                if k <= NPAIR - 3:
                    h_nxt = h_new


# --------------------------------------------------------------------------
# Entry point
# --------------------------------------------------------------------------
def kernel(**inputs):
    global LAST_EXEC_NS, LAST_RESULTS
    nc = build_program()

    W_pack = _host_weights(
        inputs["Lambda_re"], inputs["Lambda_im"], inputs["P_re"],
        inputs["P_im"], inputs["B_re"], inputs["B_im"], inputs["C_ri"],
        inputs["D"], inputs["log_step"])

    # u [BH, L] -> per-core [q, i, j, b'] bf16: uT[c, q, i*512 + j*128 + b']
    #   = u[c*512 + j*128 + b', i*128 + q]
    u = np.asarray(inputs["u"], dtype=np.float32)
    uT = np.ascontiguousarray(
        u.reshape(NCORES, 4, 128, NCH, 128).transpose(0, 4, 3, 1, 2)
    ).reshape(NCORES, 128, NCH * 512).astype(NPBF16)

    in_maps = []
    for c in range(NCORES):
        in_maps.append({"uT": uT[c], "W": W_pack})

    trace = bool(int(os.environ.get("KERNEL_TRACE", "0")))
    kw = {}
    if trace:
        kw["trace"] = True
        kw["trace_cores"] = list(range(NCORES))
    res = run_bass_kernel_spmd(nc, in_maps, list(range(NCORES)), **kw)
    LAST_EXEC_NS = res.exec_time_ns
    LAST_RESULTS = res

    # y_d rows jj*128+t, cols c4*512 + j*128 + b'
    #   -> y[core, j*128+b', (4*jj+c4)*128+t]
    outs = []
    for c in range(NCORES):
        yc = np.asarray(res.results[c]["y"])  # (1024, 2048) bf16
        yc = yc.reshape(8, 128, 4, 4, 128).transpose(3, 4, 0, 2, 1)
        outs.append(yc.reshape(BC, L).astype(np.float32))
    return np.concatenate(outs, axis=0)
